# revision 23
# baseline (speedup 1.0000x reference)
"""Trainium2 Bass kernel for nn_AlignmentMatrix.

score[b,i,j] = [ctx_i ; asp_j ; ctx_i*asp_j] @ w_u
            = sum_d ctx[b,i,d]*w3[d]*asp[b,j,d] + ctx[b]@w1 + asp[b]@w2

Reformulated per batch as a single matmul over host-marshalled operands:
    out[b] = ctxp[b] @ R[b]
with (D=400)
    ctxp[b][i, 0:400] = ctx[b][i, :]           (fp16)
    ctxp[b][i, 400]   = 1.0                     (bias lane)
    R[b][d, j] = w3[d]*asp[b,j,d] + w1[d]       (folds ctx@w1)
    R[b][400, j] = asp[b,j,:] @ w2              (folds asp@w2)
The 54.9 GFLOP contraction runs on-device with fp32 PSUM accumulation;
host prep is O(B*L*D) elementwise marshalling + layout.

Marshalling / kernel-structure choices:
  - ctx ships ALREADY TRANSPOSED (d-major) so the device does no
    transposition at all: 3 full K=128 blocks [pb, 384, 2048].
  - the contraction tail (16 dims + bias lane = K=17) ships separately,
    replicated at partition offsets {0,32,64,96}: the four row-slots of
    an output group run their tail matmuls CONCURRENTLY in one array
    pass via tile_position row tiling (saves ~3/16 of PE time vs
    padding the tail to K=128).
  - the i-axis is permuted host-side as i' = g*512 + r*128 + p
    (i = g*512 + 4p + r) so output partition p holds 4 CONSECUTIVE
    output rows -> every store descriptor is one 4KB contiguous line.
  - R ships as [pb, 128, 4, 512] partition-major (one 4KB descriptor
    per partition per batch); block 3 rows carry the tail R slices
    replicated at the same partition offsets.

Device pipeline per batch: ctx-block DMA loads (batch 0 split finer to
shorten the pipeline head), then per group: 12 full matmuls + the
4-way tail bundle -> PSUM, PSUM->SBUF fp16 copies alternating
scalar/vector engines, 4KB-per-partition stores.  The PE runs nothing
but main matmuls.  Host upcasts fp16 -> f32.
"""

import numpy as np

import concourse.bass as bass
from concourse import bacc
import concourse.mybir as mybir
import concourse.tile as tile
from concourse.bass_utils import run_bass_kernel_spmd

F32 = mybir.dt.float32
F16 = mybir.dt.float16

B, LC, LA, D = 64, 2048, 512, 400
NCH = 3           # full K=128 blocks; tail handled by the bundle
KT = 17           # tail rows: 16 data dims + bias lane
N_CORES = 8
PB = B // N_CORES  # batches per core
P = 128
RSLOT = 4          # consecutive out rows per partition
GROUP = P * RSLOT  # out rows per group (512)
NG = LC // GROUP   # groups per batch


def build_kernel(pb: int = PB, lc: int = LC) -> bass.Bass:
    nc = bacc.Bacc(
        "TRN2",
        target_bir_lowering=False,
        debug=False,
        num_devices=N_CORES,
    )
    ctx_d = nc.dram_tensor("ctx", [pb, NCH * P, lc], F16, kind="ExternalInput").ap()
    tl_d = nc.dram_tensor("tl", [pb, P, NG, P], F16, kind="ExternalInput").ap()
    rr_d = nc.dram_tensor("rr", [pb, P, NCH + 1, LA], F16, kind="ExternalInput").ap()
    out_d = nc.dram_tensor("out", [pb, lc, LA], F16, kind="ExternalOutput").ap()

    with tile.TileContext(nc) as tc:
        _kernel_body(tc, out_d, ctx_d, tl_d, rr_d, pb, lc)
    nc.compile()
    return nc


def _kernel_body(tc, out_d, ctx_d, tl_d, rr_d, pb, lc):
    nc = tc.nc

    ctx_pool = tc.alloc_tile_pool(name="ctxT", bufs=3)
    tl_pool = tc.alloc_tile_pool(name="tl", bufs=2)
    rr_pool = tc.alloc_tile_pool(name="rrt", bufs=3)
    out_pool = tc.alloc_tile_pool(name="outT", bufs=3)
    warm_pool = tc.alloc_tile_pool(name="warm", bufs=1)
    psum_o = tc.alloc_tile_pool(name="psumO", bufs=8, space="PSUM")

    # HAM warm-up: ~3.4us of dummy matmuls with no input dependencies run
    # during the input-DMA head, so the PE is at K=8/8 (2.4 GHz) when the
    # real stream starts.  Operands are zeroed scratch; output is a
    # dedicated scratch PSUM bank that is never read.
    wl = warm_pool.tile([P, P], F16, tag="wl", name="wl")
    wr = warm_pool.tile([P, P], F16, tag="wr", name="wr")
    nc.vector.memset(wl, 0.0)
    nc.vector.memset(wr, 0.0)
    pw = psum_o.tile([P, LA], F32, tag="pO", name="pw")
    for i in range(32):
        nc.tensor.matmul(pw[:, 0:P], wl, wr, start=True, stop=True)

    copy_parity = 0
    for b in range(pb):
        rrt = rr_pool.tile([P, (NCH + 1) * LA], F16, tag="rrt", name=f"rrt_{b}")
        tlt = tl_pool.tile([P, NG * P], F16, tag="tl", name=f"tl_{b}")
        ctxT = ctx_pool.tile([P, NCH * lc], F16, tag="ctxT", name=f"ctxT_{b}")
        if b == 0:
            # fine-grained first batch, interleaving ctx-block and R-block
            # pieces so group 0 slot 0's operands land in the first ~500KB
            # and the PE starts ~3us earlier
            for c in range(NCH):
                nc.gpsimd.dma_start(
                    out=ctxT[:, c * lc : c * lc + GROUP],
                    in_=ctx_d[b, c * P : (c + 1) * P, 0:GROUP],
                )
                nc.gpsimd.dma_start(
                    out=rrt[:, c * LA : (c + 1) * LA],
                    in_=rr_d[b, :, c, :],
                )
            nc.gpsimd.dma_start(
                out=rrt[:, NCH * LA : (NCH + 1) * LA],
                in_=rr_d[b, :, NCH, :],
            )
            nc.gpsimd.dma_start(
                out=tlt.rearrange("p (g q) -> p g q", g=NG),
                in_=tl_d[b],
            )
            for c in range(NCH):
                nc.gpsimd.dma_start(
                    out=ctxT[:, c * lc + GROUP : (c + 1) * lc],
                    in_=ctx_d[b, c * P : (c + 1) * P, GROUP:],
                )
        else:
            # R for batch b: [128 dd, (c, j)]; 4KB/partition contiguous.
            nc.gpsimd.dma_start(
                out=rrt.rearrange("p (c j) -> p c j", c=NCH + 1),
                in_=rr_d[b],
            )
            # tails: [128, (g, p)]; 1KB/partition contiguous
            nc.gpsimd.dma_start(
                out=tlt.rearrange("p (g q) -> p g q", g=NG),
                in_=tl_d[b],
            )
            for c in range(NCH):
                nc.gpsimd.dma_start(
                    out=ctxT[:, c * lc : (c + 1) * lc],
                    in_=ctx_d[b, c * P : (c + 1) * P, :],
                )

        for g in range(NG):
            i0 = g * GROUP
            ot = out_pool.tile([P, RSLOT * LA], F16, tag="ot",
                               name=f"ot_{b}_{g}")
            psl = [
                psum_o.tile([P, LA], F32, tag="pO", name=f"pO_{b}_{g}_{r}")
                for r in range(RSLOT)
            ]
            # concurrent tail bundle FIRST: slot r's K=17 matmul in array
            # rows [32r, 32r+17) -- all four share one streaming pass.
            # Leading with it staggers the slots' completion so copies
            # and the store drain overlap the remaining matmuls.
            for r in range(RSLOT):
                nc.tensor.matmul(
                    psl[r],
                    tlt[32 * r : 32 * r + KT, g * P : (g + 1) * P],
                    rrt[32 * r : 32 * r + KT, NCH * LA : (NCH + 1) * LA],
                    start=True,
                    stop=False,
                    tile_position=(32 * r, 0),
                )
            for r in range(RSLOT):
                pO = psl[r]
                for c in range(NCH):
                    col = c * lc + i0 + r * P
                    nc.tensor.matmul(
                        pO,
                        ctxT[:, col : col + P],
                        rrt[:, c * LA : (c + 1) * LA],
                        start=False,
                        stop=(c == NCH - 1),
                    )
                dst = ot[:, r * LA : (r + 1) * LA]
                if copy_parity & 1:
                    nc.vector.tensor_copy(dst, pO)
                else:
                    nc.scalar.copy(dst, pO)
                copy_parity += 1

            # store: partition p -> rows i0 + 4p + r, 4KB contiguous.
            # sync only: it has no compute, so its blocking semaphore
            # wait on ot can't stall anything else.  The final group is
            # split in half so its drain starts ~1us earlier.
            if b == pb - 1 and g == NG - 1:
                for h in range(RSLOT):
                    nc.sync.dma_start(
                        out=out_d[
                            b, i0 : i0 + GROUP, :
                        ].rearrange("(p r) j -> p r j", p=P)[
                            :, h : h + 1, :
                        ],
                        in_=ot.rearrange("p (r j) -> p r j", r=RSLOT)[
                            :, h : h + 1, :
                        ],
                    )
            else:
                nc.sync.dma_start(
                    out=out_d[b, i0 : i0 + GROUP, :].rearrange(
                        "(p r) j -> p r j", p=P
                    ),
                    in_=ot.rearrange("p (r j) -> p r j", r=RSLOT),
                )

    for p in reversed((ctx_pool, tl_pool, rr_pool, out_pool, warm_pool,
                       psum_o)):
        p.release()


def _prep_inputs(ctx, asp, w_u):
    """Host-side marshalling: fp16 cast, transpose/permute, R formation."""
    ctx = np.asarray(ctx, dtype=np.float32)
    asp = np.asarray(asp, dtype=np.float32)
    w = np.asarray(w_u, dtype=np.float32).reshape(-1)
    w1, w2, w3 = w[:D], w[D : 2 * D], w[2 * D :]

    # ctxT (first 384 dims) with i' = g*512 + r*128 + p <-> i = g*512+4p+r
    # [B, i, d] -> [B, d, g, p, r] -> [B, d, g, r, p]
    cr = ctx.reshape(B, NG, P, RSLOT, D)
    ctxp = (
        np.transpose(cr[..., : NCH * P], (0, 4, 1, 3, 2))
        .reshape(B, NCH * P, LC)
        .astype(np.float16)
    )

    # tails: [B, 128, g, p]; partition 32r+t holds tail dim t (t<16) or
    # the bias lane (t=16) for slot r; column (g, p) is out row g*512+4p+r
    tails = np.zeros((B, P, NG, P), dtype=np.float16)
    tail_d = np.transpose(cr[..., NCH * P :], (0, 4, 1, 2, 3))  # [B,16,g,p,r]
    for r in range(RSLOT):
        tails[:, 32 * r : 32 * r + 16, :, :] = tail_d[..., r]
        tails[:, 32 * r + 16, :, :] = 1.0

    # R[b, dd, c, j]: blocks 0..2 rows dd -> d = 128c + dd; block 3 rows
    # 32r+t -> tail slice (replicated for each slot offset)
    scaled = (asp * w3[None, None, :] + w1[None, None, :]).transpose(0, 2, 1)
    at = asp @ w2
    rr = np.zeros((B, P, NCH + 1, LA), dtype=np.float16)
    for c in range(NCH):
        rr[:, :, c, :] = scaled[:, P * c : P * (c + 1), :]
    for r in range(RSLOT):
        rr[:, 32 * r : 32 * r + 16, NCH, :] = scaled[:, NCH * P :, :]
        rr[:, 32 * r + 16, NCH, :] = at
    return ctxp, tails, rr


def kernel(batch_size=None, ctx=None, asp=None, w_u=None, **_unused):
    ctxp, tails, rr = _prep_inputs(ctx, asp, w_u)

    nc = build_kernel()
    in_maps = [
        {
            "ctx": ctxp[i * PB : (i + 1) * PB],
            "tl": tails[i * PB : (i + 1) * PB],
            "rr": rr[i * PB : (i + 1) * PB],
        }
        for i in range(N_CORES)
    ]
    res = run_bass_kernel_spmd(
        nc, in_maps, core_ids=list(range(N_CORES)), **_RUN_KWARGS
    )
    _LAST_RESULTS.clear()
    _LAST_RESULTS.append(res)
    # stores write natural row order (partition p, slot r -> row 4p+r)
    out = np.concatenate(
        [np.asarray(res.results[i]["out"]) for i in range(N_CORES)], axis=0
    )
    return out.astype(np.float32)


# test-harness hooks: extra kwargs for run_bass_kernel_spmd (e.g. trace=True)
# and the last BassKernelResults for profiling. Unused in grading.
_RUN_KWARGS: dict = {}
_LAST_RESULTS: list = []


# revision 24
# speedup vs baseline: 1.0576x; 1.0576x over previous
"""Trainium2 Bass kernel for nn_AlignmentMatrix.

score[b,i,j] = [ctx_i ; asp_j ; ctx_i*asp_j] @ w_u
            = sum_d ctx[b,i,d]*w3[d]*asp[b,j,d] + ctx[b]@w1 + asp[b]@w2

Reformulated per batch as a single matmul over host-marshalled operands:
    out[b] = ctxp[b] @ R[b]
with (D=400)
    ctxp[b][i, 0:400] = ctx[b][i, :]           (fp16)
    ctxp[b][i, 400]   = 1.0                     (bias lane)
    R[b][d, j] = w3[d]*asp[b,j,d] + w1[d]       (folds ctx@w1)
    R[b][400, j] = asp[b,j,:] @ w2              (folds asp@w2)
The 54.9 GFLOP contraction runs on-device with fp32 PSUM accumulation;
host prep is O(B*L*D) elementwise marshalling + layout.

Marshalling / kernel-structure choices:
  - ctx ships ALREADY TRANSPOSED (d-major) so the device does no
    transposition at all: 3 full K=128 blocks [pb, 384, 2048].
  - the contraction tail (16 dims + bias lane = K=17) ships separately,
    replicated at partition offsets {0,32,64,96}: the four row-slots of
    an output group run their tail matmuls CONCURRENTLY in one array
    pass via tile_position row tiling (saves ~3/16 of PE time vs
    padding the tail to K=128).
  - the i-axis is permuted host-side as i' = g*512 + r*128 + p
    (i = g*512 + 4p + r) so output partition p holds 4 CONSECUTIVE
    output rows -> every store descriptor is one 4KB contiguous line.
  - R ships as [pb, 128, 4, 512] partition-major (one 4KB descriptor
    per partition per batch); block 3 rows carry the tail R slices
    replicated at the same partition offsets.

Device pipeline per batch: ctx-block DMA loads (batch 0 split finer to
shorten the pipeline head), then per group: 12 full matmuls + the
4-way tail bundle -> PSUM, PSUM->SBUF fp16 copies alternating
scalar/vector engines, 4KB-per-partition stores.  The PE runs nothing
but main matmuls.  Host upcasts fp16 -> f32.
"""

import numpy as np

import concourse.bass as bass
from concourse import bacc
import concourse.mybir as mybir
import concourse.tile as tile
from concourse.bass_utils import run_bass_kernel_spmd

F32 = mybir.dt.float32
F16 = mybir.dt.float16

B, LC, LA, D = 64, 2048, 512, 400
NCH = 3           # full K=128 blocks; tail handled by the bundle
KT = 17           # tail rows: 16 data dims + bias lane
N_CORES = 8
PB = B // N_CORES  # batches per core
P = 128
RSLOT = 4          # consecutive out rows per partition
GROUP = P * RSLOT  # out rows per group (512)
NG = LC // GROUP   # groups per batch


def build_kernel(pb: int = PB, lc: int = LC) -> bass.Bass:
    nc = bacc.Bacc(
        "TRN2",
        target_bir_lowering=False,
        debug=False,
        num_devices=N_CORES,
    )
    ctx_d = nc.dram_tensor("ctx", [pb, NCH * P, lc], F16, kind="ExternalInput").ap()
    tl_d = nc.dram_tensor("tl", [pb, P, NG, P], F16, kind="ExternalInput").ap()
    rr_d = nc.dram_tensor("rr", [pb, P, NCH + 1, LA], F16, kind="ExternalInput").ap()
    out_d = nc.dram_tensor("out", [pb, lc, LA], F16, kind="ExternalOutput").ap()

    with tile.TileContext(nc) as tc:
        _kernel_body(tc, out_d, ctx_d, tl_d, rr_d, pb, lc)
    nc.compile()
    return nc


def _kernel_body(tc, out_d, ctx_d, tl_d, rr_d, pb, lc):
    nc = tc.nc

    ctx_pool = tc.alloc_tile_pool(name="ctxT", bufs=3)
    tl_pool = tc.alloc_tile_pool(name="tl", bufs=2)
    rr_pool = tc.alloc_tile_pool(name="rrt", bufs=3)
    out_pool = tc.alloc_tile_pool(name="outT", bufs=3)
    warm_pool = tc.alloc_tile_pool(name="warm", bufs=1)
    psum_o = tc.alloc_tile_pool(name="psumO", bufs=8, space="PSUM")

    # HAM warm-up: ~3.4us of dummy matmuls with no input dependencies run
    # during the input-DMA head, so the PE is at K=8/8 (2.4 GHz) when the
    # real stream starts.  Operands are zeroed scratch; output is a
    # dedicated scratch PSUM bank that is never read.
    wl = warm_pool.tile([P, P], F16, tag="wl", name="wl")
    wr = warm_pool.tile([P, P], F16, tag="wr", name="wr")
    nc.vector.memset(wl, 0.0)
    nc.vector.memset(wr, 0.0)
    pw = psum_o.tile([P, LA], F32, tag="pO", name="pw")
    for i in range(32):
        nc.tensor.matmul(pw[:, 0:P], wl, wr, start=True, stop=True)

    copy_parity = 0
    for b in range(pb):
        rrt = rr_pool.tile([P, (NCH + 1) * LA], F16, tag="rrt", name=f"rrt_{b}")
        tlt = tl_pool.tile([P, NG * P], F16, tag="tl", name=f"tl_{b}")
        ctxT = ctx_pool.tile([P, NCH * lc], F16, tag="ctxT", name=f"ctxT_{b}")
        if b == 0:
            # fine-grained first batch, interleaving ctx-block and R-block
            # pieces so group 0 slot 0's operands land in the first ~500KB
            # and the PE starts ~3us earlier
            for c in range(NCH):
                nc.gpsimd.dma_start(
                    out=ctxT[:, c * lc : c * lc + GROUP],
                    in_=ctx_d[b, c * P : (c + 1) * P, 0:GROUP],
                )
                nc.gpsimd.dma_start(
                    out=rrt[:, c * LA : (c + 1) * LA],
                    in_=rr_d[b, :, c, :],
                )
            nc.gpsimd.dma_start(
                out=rrt[:, NCH * LA : (NCH + 1) * LA],
                in_=rr_d[b, :, NCH, :],
            )
            nc.gpsimd.dma_start(
                out=tlt.rearrange("p (g q) -> p g q", g=NG),
                in_=tl_d[b],
            )
            for c in range(NCH):
                nc.gpsimd.dma_start(
                    out=ctxT[:, c * lc + GROUP : (c + 1) * lc],
                    in_=ctx_d[b, c * P : (c + 1) * P, GROUP:],
                )
        else:
            # R for batch b: [128 dd, (c, j)]; 4KB/partition contiguous.
            nc.gpsimd.dma_start(
                out=rrt.rearrange("p (c j) -> p c j", c=NCH + 1),
                in_=rr_d[b],
            )
            # tails: [128, (g, p)]; 1KB/partition contiguous
            nc.gpsimd.dma_start(
                out=tlt.rearrange("p (g q) -> p g q", g=NG),
                in_=tl_d[b],
            )
            for c in range(NCH):
                nc.gpsimd.dma_start(
                    out=ctxT[:, c * lc : (c + 1) * lc],
                    in_=ctx_d[b, c * P : (c + 1) * P, :],
                )

        for g in range(NG):
            i0 = g * GROUP
            ot = out_pool.tile([P, RSLOT * LA], F16, tag="ot",
                               name=f"ot_{b}_{g}")
            psl = [
                psum_o.tile([P, LA], F32, tag="pO", name=f"pO_{b}_{g}_{r}")
                for r in range(RSLOT)
            ]
            for r in range(RSLOT):
                pO = psl[r]
                for c in range(NCH):
                    col = c * lc + i0 + r * P
                    nc.tensor.matmul(
                        pO,
                        ctxT[:, col : col + P],
                        rrt[:, c * LA : (c + 1) * LA],
                        start=(c == 0),
                        stop=False,
                    )
            # concurrent tail bundle: slot r's K=17 matmul in array rows
            # [32r, 32r+17) -- all four share one streaming pass
            for r in range(RSLOT):
                nc.tensor.matmul(
                    psl[r],
                    tlt[32 * r : 32 * r + KT, g * P : (g + 1) * P],
                    rrt[32 * r : 32 * r + KT, NCH * LA : (NCH + 1) * LA],
                    start=False,
                    stop=True,
                    tile_position=(32 * r, 0),
                )
            for r in range(RSLOT):
                dst = ot[:, r * LA : (r + 1) * LA]
                if copy_parity & 1:
                    nc.vector.tensor_copy(dst, psl[r])
                else:
                    nc.scalar.copy(dst, psl[r])
                copy_parity += 1

            # store: partition p -> rows i0 + 4p + r, 4KB contiguous.
            # sync only: it has no compute, so its blocking semaphore
            # wait on ot can't stall anything else.  The final group is
            # split in half so its drain starts ~1us earlier.
            if b == pb - 1 and g == NG - 1:
                for h in range(RSLOT):
                    nc.sync.dma_start(
                        out=out_d[
                            b, i0 : i0 + GROUP, :
                        ].rearrange("(p r) j -> p r j", p=P)[
                            :, h : h + 1, :
                        ],
                        in_=ot.rearrange("p (r j) -> p r j", r=RSLOT)[
                            :, h : h + 1, :
                        ],
                    )
            else:
                nc.sync.dma_start(
                    out=out_d[b, i0 : i0 + GROUP, :].rearrange(
                        "(p r) j -> p r j", p=P
                    ),
                    in_=ot.rearrange("p (r j) -> p r j", r=RSLOT),
                )

    for p in reversed((ctx_pool, tl_pool, rr_pool, out_pool, warm_pool,
                       psum_o)):
        p.release()


def _prep_inputs(ctx, asp, w_u):
    """Host-side marshalling: fp16 cast, transpose/permute, R formation."""
    ctx = np.asarray(ctx, dtype=np.float32)
    asp = np.asarray(asp, dtype=np.float32)
    w = np.asarray(w_u, dtype=np.float32).reshape(-1)
    w1, w2, w3 = w[:D], w[D : 2 * D], w[2 * D :]

    # ctxT (first 384 dims) with i' = g*512 + r*128 + p <-> i = g*512+4p+r
    # [B, i, d] -> [B, d, g, p, r] -> [B, d, g, r, p]
    cr = ctx.reshape(B, NG, P, RSLOT, D)
    ctxp = (
        np.transpose(cr[..., : NCH * P], (0, 4, 1, 3, 2))
        .reshape(B, NCH * P, LC)
        .astype(np.float16)
    )

    # tails: [B, 128, g, p]; partition 32r+t holds tail dim t (t<16) or
    # the bias lane (t=16) for slot r; column (g, p) is out row g*512+4p+r
    tails = np.zeros((B, P, NG, P), dtype=np.float16)
    tail_d = np.transpose(cr[..., NCH * P :], (0, 4, 1, 2, 3))  # [B,16,g,p,r]
    for r in range(RSLOT):
        tails[:, 32 * r : 32 * r + 16, :, :] = tail_d[..., r]
        tails[:, 32 * r + 16, :, :] = 1.0

    # R[b, dd, c, j]: blocks 0..2 rows dd -> d = 128c + dd; block 3 rows
    # 32r+t -> tail slice (replicated for each slot offset)
    scaled = (asp * w3[None, None, :] + w1[None, None, :]).transpose(0, 2, 1)
    at = asp @ w2
    rr = np.zeros((B, P, NCH + 1, LA), dtype=np.float16)
    for c in range(NCH):
        rr[:, :, c, :] = scaled[:, P * c : P * (c + 1), :]
    for r in range(RSLOT):
        rr[:, 32 * r : 32 * r + 16, NCH, :] = scaled[:, NCH * P :, :]
        rr[:, 32 * r + 16, NCH, :] = at
    return ctxp, tails, rr


def kernel(batch_size=None, ctx=None, asp=None, w_u=None, **_unused):
    ctxp, tails, rr = _prep_inputs(ctx, asp, w_u)

    nc = build_kernel()
    in_maps = [
        {
            "ctx": ctxp[i * PB : (i + 1) * PB],
            "tl": tails[i * PB : (i + 1) * PB],
            "rr": rr[i * PB : (i + 1) * PB],
        }
        for i in range(N_CORES)
    ]
    res = run_bass_kernel_spmd(
        nc, in_maps, core_ids=list(range(N_CORES)), **_RUN_KWARGS
    )
    _LAST_RESULTS.clear()
    _LAST_RESULTS.append(res)
    # stores write natural row order (partition p, slot r -> row 4p+r)
    out = np.concatenate(
        [np.asarray(res.results[i]["out"]) for i in range(N_CORES)], axis=0
    )
    return out.astype(np.float32)


# test-harness hooks: extra kwargs for run_bass_kernel_spmd (e.g. trace=True)
# and the last BassKernelResults for profiling. Unused in grading.
_RUN_KWARGS: dict = {}
_LAST_RESULTS: list = []
